# revision 1
# baseline (speedup 1.0000x reference)
"""Cross-attention transformer block on 8 TRN2 NeuronCores.

Sharding: 8 cores = 4 batches x 2 sequence-halves. Core c handles batch
b = c//2, query tokens [hf*1024, (hf+1)*1024) with hf = c%2. Each core
computes the FULL kv projection for its batch (duplicated across the 2
cores of a batch) so no collectives are needed.

Layout strategy: everything "feature-major" (transposed, [feature, token])
so every matmul contracts over the partition dim with natural weight
layouts and no on-chip transposes of big activations:
  x_hat^T = LN(x^T)          (stats via ones-matmul column sums)
  q^T = Wq^T @ x_hat         (lhsT = Wq chunk, rhs = x_hat^T), rope fused
  k^T = Wk^T @ c_hat         rope fused
  v   = c_hat @ Wv           (lhsT = c_hat^T chunk)
  scores^T[k,q] = (k^T slice).T @ q^T slice        per head, K=64
  e = exp(scores/8)          (no max subtraction; scores are O(1))
  U^T: lhsT = v_ext[k,65] -> rows 0..63 = attn numerator,
       row 64 = softmax denominator (ones column folded into v)
  attn^T = U^T[0:64] * (1/U^T[64]) broadcast via K=1 matmul
  x2^T = Wo^T @ attn + x^T + bo
  x3^T = W2^T @ gelu(W1^T @ LN(x2^T) + b1) + x2^T + b2
Matmuls in bf16 (weights cast host-side), accumulation fp32, LN/residual
paths fp32, LN stat matmuls in float32r.
"""

import numpy as np
import ml_dtypes

import concourse.bass as bass
import concourse.bacc as bacc
import concourse.mybir as mybir
import concourse.tile as tile
from concourse.bass_utils import run_bass_kernel_spmd

F32 = mybir.dt.float32
F32R = mybir.dt.float32r
BF16 = mybir.dt.bfloat16
AF = mybir.ActivationFunctionType
ALU = mybir.AluOpType

B, L, D, H, HD = 4, 2048, 1024, 16, 64
TQ = 1024          # query tokens per core
TK = 2048          # kv tokens per core
HID = 4 * D
NCORES = 8
P = 128
DC = D // P        # 8 feature chunks
KC = TK // P       # 16 kv-token chunks
NHC = HID // P     # 32 hidden chunks
EPS = 1e-5

# vecs[:, i, :] packing indices
(V_BQ, V_BK, V_BO, V_B2, V_GQ, V_BEQ, V_GKV, V_BEKV, V_GFFN, V_BEFFN,
 V_BQR, V_BKR) = range(12)

_CACHED_NC = None


def _pool(tc, name, bufs, side="left"):
    cm = tc.tile_pool(name=name, bufs=bufs, side=side)
    return cm, cm.__enter__()


def build_nc():
    nc = bacc.Bacc("TRN2", debug=False, num_devices=NCORES)

    xT = nc.declare_dram_parameter("xT", [D, TQ], F32, False).ap()
    ctxT = nc.declare_dram_parameter("ctxT", [D, TK], F32, False).ap()
    cosq = nc.declare_dram_parameter("cosq", [P, TQ], F32, False).ap()
    sinq = nc.declare_dram_parameter("sinq", [P, TQ], F32, False).ap()
    cosk = nc.declare_dram_parameter("cosk", [P, TK], F32, False).ap()
    sink = nc.declare_dram_parameter("sink", [P, TK], F32, False).ap()
    wq = nc.declare_dram_parameter("wq", [D, D], BF16, False).ap()
    wk = nc.declare_dram_parameter("wk", [D, D], BF16, False).ap()
    wv = nc.declare_dram_parameter("wv", [D, D], BF16, False).ap()
    wo = nc.declare_dram_parameter("wo", [D, D], BF16, False).ap()
    w1 = nc.declare_dram_parameter("w1", [D, HID], BF16, False).ap()
    w2 = nc.declare_dram_parameter("w2", [HID, D], BF16, False).ap()
    vecs_d = nc.declare_dram_parameter("vecs", [P, 12, DC], F32, False).ap()
    b1t_d = nc.declare_dram_parameter("b1t", [P, NHC], F32, False).ap()
    bvrow_d = nc.declare_dram_parameter("bvrow", [1, D], F32, False).ap()
    outT = nc.declare_dram_parameter("outT", [D, TQ], F32, True).ap()

    with tile.TileContext(nc) as tc:
        const_cm, const = _pool(tc, "const", 1)
        work_cm, work = _pool(tc, "work", 6)       # f32 [128,512] scratch
        stat_cm, stat = _pool(tc, "stat", 2)
        ps_cm = tc.tile_pool(name="ps", bufs=4, space="PSUM")
        ps = ps_cm.__enter__()

        # ---- constants ----
        vecs = const.tile([P, 12, DC], F32, tag="vecs")
        nc.sync.dma_start(vecs[:], vecs_d)
        b1t = const.tile([P, NHC], F32, tag="b1t")
        nc.sync.dma_start(b1t[:], b1t_d)
        bvrow = const.tile([1, D], F32, tag="bvrow")
        nc.sync.dma_start(bvrow[:], bvrow_d)
        bvb = const.tile([P, D], F32, tag="bvb")
        nc.gpsimd.partition_broadcast(bvb[:], bvrow[:])
        onesP = const.tile([P, 1], F32, tag="onesP")
        nc.vector.memset(onesP[:], 1.0)
        ones1 = const.tile([1, P], F32, tag="ones1")
        nc.vector.memset(ones1[:], 1.0)
        ones_bf = const.tile([1, HD], BF16, tag="onesbf")
        nc.vector.memset(ones_bf[:], 1.0)
        eps1 = const.tile([1, 1], F32, tag="eps1")
        nc.vector.memset(eps1[:], EPS)

        def scratch(name):
            return work.tile([P, 512], F32, tag="scratch", name=name)

        def gcol(idx, dc, pr=None):
            # [*,1] per-partition scalar column from vecs
            if pr is None:
                return vecs[:, idx, dc : dc + 1]
            return vecs[pr[0] : pr[1], idx, dc : dc + 1]

        def ln_T(load_fn, nt, g_idx, b_idx, out_pool, out_tag):
            """LayerNorm over feature dim of feature-major data.

            load_fn(dc, tt) -> [128, 512] f32 AP for feature chunk dc,
            token tile tt. Returns DC [128, nt] bf16 tiles
            = (x - mu) * rsqrt(var+eps) * g + b.
            """
            outs = [out_pool.tile([P, nt], BF16, tag=out_tag,
                                  name=f"{out_tag}{i}") for i in range(DC)]
            for tt in range(nt // 512):
                sl = slice(tt * 512, (tt + 1) * 512)
                srcs = [load_fn(dc, tt) for dc in range(DC)]
                pr_row = ps.tile([P, 512], F32, tag="row", bufs=2,
                                 name=f"lnrow_{out_tag}_{tt}")
                # sum on partition 0, sumsq on partition 32 (same bank)
                for dc in range(DC):
                    sq = scratch(f"sq_{out_tag}_{tt}_{dc}")
                    nc.scalar.square(sq[:], srcs[dc])
                    nc.tensor.matmul(
                        pr_row[0:1, :], onesP[:],
                        srcs[dc],
                        start=(dc == 0), stop=(dc == DC - 1),
                    )
                    nc.tensor.matmul(
                        pr_row[32:33, :], onesP[:],
                        sq[:],
                        start=(dc == 0), stop=(dc == DC - 1),
                    )
                st = stat.tile([1, 4, 512], F32, tag="stats",
                               name=f"st_{out_tag}_{tt}")
                mu, var, tmp, rs = (st[:, i, :] for i in range(4))
                nc.vector.tensor_scalar_mul(mu, pr_row[0:1, :], 1.0 / D)
                nc.vector.tensor_scalar_mul(var, pr_row[32:33, :], 1.0 / D)
                nc.vector.tensor_mul(tmp, mu, mu)
                nc.vector.tensor_sub(var, var, tmp)
                # tmp <- sqrt(var+eps); rs = 1/tmp
                nc.scalar.activation(tmp, var, AF.Sqrt, bias=eps1[:])
                nc.vector.reciprocal(rs, tmp)
                # broadcast mu, rs across partitions via K=1 matmul
                ps_mu = ps.tile([P, 512], F32, tag="mm", bufs=4,
                                name=f"psmu_{out_tag}_{tt}")
                nc.tensor.matmul(ps_mu[:], ones1[:],
                                 mu, start=True, stop=True)
                ps_rs = ps.tile([P, 512], F32, tag="mm", bufs=4,
                                name=f"psrs_{out_tag}_{tt}")
                nc.tensor.matmul(ps_rs[:], ones1[:],
                                 rs, start=True, stop=True)
                for dc in range(DC):
                    t = scratch(f"lnt_{out_tag}_{tt}_{dc}")
                    nc.vector.tensor_sub(t[:], srcs[dc], ps_mu[:])
                    nc.vector.tensor_mul(t[:], t[:], ps_rs[:])
                    nc.vector.tensor_scalar(
                        out=outs[dc][:, sl], in0=t[:],
                        scalar1=gcol(g_idx, dc), scalar2=gcol(b_idx, dc),
                        op0=ALU.mult, op1=ALU.add,
                    )
            return outs

        def rope_evict(psum, out_ap, cos_t, sin_t, sl, b_idx, b_rot_idx, fc):
            """out = (psum + b) * cos + rot(psum + b) * sin  (bf16 out)."""
            t = scratch(f"ropeA_{b_idx}_{fc}_{sl.start}")
            nc.vector.scalar_tensor_tensor(
                t[:], psum[:], gcol(b_idx, fc), cos_t[:, sl],
                ALU.add, ALU.mult,
            )
            t2 = scratch(f"ropeB_{b_idx}_{fc}_{sl.start}")
            for blk in range(2):   # two 64-row head blocks
                b0 = blk * 64
                for half in range(2):
                    od = slice(b0 + half * 32, b0 + half * 32 + 32)
                    sr = slice(b0 + (1 - half) * 32, b0 + (1 - half) * 32 + 32)
                    nc.vector.scalar_tensor_tensor(
                        t2[od, :], psum[sr, :],
                        gcol(b_rot_idx, fc, (od.start, od.stop)),
                        sin_t[od.start : od.stop, sl],
                        ALU.add, ALU.mult,
                    )
            nc.vector.tensor_add(out_ap, t[:], t2[:])

        def dram_loader(pool, dram_ap, tag):
            def load(dc, tt):
                t = pool.tile([P, 512], F32, tag=tag,
                              name=f"{tag}_{dc}_{tt}")
                nc.sync.dma_start(
                    t[:], dram_ap[dc * P : (dc + 1) * P,
                                  tt * 512 : (tt + 1) * 512])
                return t[:]
            return load

        # ================= phase 1a: LN(context) =================
        chat_cm, chat_p = _pool(tc, "chat", DC, side="right")
        cin_cm, cin_p = _pool(tc, "cin", 12)
        chatT = ln_T(dram_loader(cin_p, ctxT, "cin"), TK,
                     V_GKV, V_BEKV, chat_p, "chat")
        cin_cm.__exit__(None, None, None)

        # ================= phase 1b: LN(x) =================
        qT_cm, qT_p = _pool(tc, "qT", DC)
        xhat_cm, xhat_p = _pool(tc, "xhat", DC)
        xin_cm, xin_p = _pool(tc, "xin", 12)
        xhatT = ln_T(dram_loader(xin_p, xT, "xin"), TQ,
                     V_GQ, V_BEQ, xhat_p, "xhat")
        xin_cm.__exit__(None, None, None)

        # ================= phase 2: q^T with rope =================
        w_cm, w_p = _pool(tc, "wqkv", 8, side="right")
        rope_cm, rope_p = _pool(tc, "rope", 1, side="right")
        cosq_t = rope_p.tile([P, TQ], F32, tag="cosq")
        nc.sync.dma_start(cosq_t[:], cosq)
        sinq_t = rope_p.tile([P, TQ], F32, tag="sinq")
        nc.sync.dma_start(sinq_t[:], sinq)
        cosk_t = rope_p.tile([P, TK], F32, tag="cosk")
        nc.sync.dma_start(cosk_t[:], cosk)
        sink_t = rope_p.tile([P, TK], F32, tag="sink")
        nc.sync.dma_start(sink_t[:], sink)

        wq_sb = []
        for dc in range(DC):
            wt = w_p.tile([P, D], BF16, tag="w", name=f"wq{dc}")
            nc.sync.dma_start(wt[:], wq[dc * P : (dc + 1) * P, :])
            wq_sb.append(wt)
        qT = [qT_p.tile([P, TQ], BF16, tag="qT", name=f"qT{i}")
              for i in range(DC)]
        for fc in range(DC):
            for tt in range(TQ // 512):
                sl = slice(tt * 512, (tt + 1) * 512)
                pm = ps.tile([P, 512], F32, tag="mm", bufs=4,
                             name=f"pmq_{fc}_{tt}")
                for dc in range(DC):
                    nc.tensor.matmul(
                        pm[:], wq_sb[dc][:, fc * P : (fc + 1) * P],
                        xhatT[dc][:, sl], start=(dc == 0), stop=(dc == DC - 1),
                    )
                rope_evict(pm, qT[fc][:, sl], cosq_t, sinq_t, sl, V_BQ, V_BQR, fc)
        xhat_cm.__exit__(None, None, None)

        # ================= phase 3: k^T with rope =================
        kT_cm, kT_p = _pool(tc, "kT", DC)
        wk_sb = []
        for dc in range(DC):
            wt = w_p.tile([P, D], BF16, tag="w", name=f"wk{dc}")
            nc.sync.dma_start(wt[:], wk[dc * P : (dc + 1) * P, :])
            wk_sb.append(wt)
        kT = [kT_p.tile([P, TK], BF16, tag="kT", name=f"kT{i}")
              for i in range(DC)]
        for fc in range(DC):
            for tt in range(TK // 512):
                sl = slice(tt * 512, (tt + 1) * 512)
                pm = ps.tile([P, 512], F32, tag="mm", bufs=4,
                             name=f"pmk_{fc}_{tt}")
                for dc in range(DC):
                    nc.tensor.matmul(
                        pm[:], wk_sb[dc][:, fc * P : (fc + 1) * P],
                        chatT[dc][:, sl], start=(dc == 0), stop=(dc == DC - 1),
                    )
                rope_evict(pm, kT[fc][:, sl], cosk_t, sink_t, sl, V_BK, V_BKR, fc)
        rope_cm.__exit__(None, None, None)

        # ================= phase 4: v_ext (natural layout + ones col) ===
        vext_cm, vext_p = _pool(tc, "vext", KC)
        wv_sb = []
        for dc in range(DC):
            wt = w_p.tile([P, D], BF16, tag="w", name=f"wv{dc}")
            nc.sync.dma_start(wt[:], wv[dc * P : (dc + 1) * P, :])
            wv_sb.append(wt)
        vext = []
        for kc in range(KC):
            vt = vext_p.tile([P, H, HD + 1], BF16, tag="vext",
                             name=f"vext{kc}")
            nc.vector.memset(vt[:, :, HD : HD + 1], 1.0)
            vext.append(vt)
        for kc in range(KC):
            for f2 in range(2):
                pm = ps.tile([P, 512], F32, tag="mm", bufs=4,
                             name=f"pmv_{kc}_{f2}")
                for dc in range(DC):
                    nc.tensor.matmul(
                        pm[:], chatT[dc][:, kc * P : (kc + 1) * P],
                        wv_sb[dc][:, f2 * 512 : (f2 + 1) * 512],
                        start=(dc == 0), stop=(dc == DC - 1),
                    )
                nc.vector.tensor_add(
                    vext[kc][:, f2 * 8 : (f2 + 1) * 8, 0:HD],
                    pm[:].rearrange("p (h d) -> p h d", d=HD),
                    bvb[:, f2 * 512 : (f2 + 1) * 512].rearrange(
                        "p (h d) -> p h d", d=HD),
                )
        w_cm.__exit__(None, None, None)
        chat_cm.__exit__(None, None, None)

        # ================= phase 5: attention =================
        attnT_cm, attnT_p = _pool(tc, "attnT", DC, side="right")
        e_cm, e_p = _pool(tc, "epool", 6)
        attnT = [attnT_p.tile([P, TQ], BF16, tag="attnT", name=f"attnT{i}")
                 for i in range(DC)]
        for tt in range(TQ // 512):
            sl = slice(tt * 512, (tt + 1) * 512)
            for h in range(H):
                fc, hb = h // 2, (h % 2) * HD
                pu = ps.tile([P, 512], F32, tag="u", bufs=2,
                             name=f"pu_{tt}_{h}")
                for kc in range(KC):
                    pscore = ps.tile([P, 512], F32, tag="mm", bufs=4,
                                     name=f"pscore_{tt}_{h}_{kc}")
                    nc.tensor.matmul(
                        pscore[:],
                        kT[fc][hb : hb + HD, kc * P : (kc + 1) * P],
                        qT[fc][hb : hb + HD, sl],
                        start=True, stop=True,
                    )
                    e = e_p.tile([P, 512], BF16, tag="e",
                                 name=f"e_{tt}_{h}_{kc}")
                    nc.scalar.activation(e[:], pscore[:], AF.Exp, scale=0.125)
                    nc.tensor.matmul(
                        pu[0 : HD + 1, :],
                        vext[kc][:, h, :],
                        e[:], start=(kc == 0), stop=(kc == KC - 1),
                    )
                rcp = stat.tile([1, 512], F32, tag="rcp",
                                name=f"rcp_{tt}_{h}")
                nc.vector.reciprocal(rcp[:], pu[HD : HD + 1, :])
                rb = scratch(f"rb_{tt}_{h}")
                nc.gpsimd.partition_broadcast(rb[:], rcp[:])
                nc.vector.tensor_mul(
                    attnT[fc][hb : hb + HD, sl], pu[0:HD, :],
                    rb[hb : hb + HD, :])
        e_cm.__exit__(None, None, None)
        vext_cm.__exit__(None, None, None)
        kT_cm.__exit__(None, None, None)
        qT_cm.__exit__(None, None, None)

        # ================= phase 6: x2^T = Wo^T attn + x^T + bo ========
        wo_cm, wo_p = _pool(tc, "wo", DC, side="right")
        x2_cm, x2_p = _pool(tc, "x2", DC)
        xin6_cm, xin6_p = _pool(tc, "xin6", 3, side="right")
        wo_sb = []
        for dc in range(DC):
            wt = wo_p.tile([P, D], BF16, tag="wo", name=f"wo{dc}")
            nc.sync.dma_start(wt[:], wo[dc * P : (dc + 1) * P, :])
            wo_sb.append(wt)
        x2T = [x2_p.tile([P, TQ], F32, tag="x2", name=f"x2T{i}")
               for i in range(DC)]
        for fc in range(DC):
            for tt in range(TQ // 512):
                sl = slice(tt * 512, (tt + 1) * 512)
                xres = xin6_p.tile([P, 512], F32, tag="xin6",
                                   name=f"xres_{fc}_{tt}")
                nc.sync.dma_start(xres[:], xT[fc * P : (fc + 1) * P, sl])
                pm = ps.tile([P, 512], F32, tag="mm", bufs=4,
                             name=f"pmo_{fc}_{tt}")
                for dc in range(DC):
                    nc.tensor.matmul(
                        pm[:], wo_sb[dc][:, fc * P : (fc + 1) * P],
                        attnT[dc][:, sl], start=(dc == 0), stop=(dc == DC - 1),
                    )
                nc.vector.scalar_tensor_tensor(
                    x2T[fc][:, sl], pm[:], gcol(V_BO, fc), xres[:],
                    ALU.add, ALU.add,
                )
        xin6_cm.__exit__(None, None, None)
        wo_cm.__exit__(None, None, None)
        attnT_cm.__exit__(None, None, None)

        # ================= phase 7: LN(x2) =================
        xhat2_cm, xhat2_p = _pool(tc, "xhat2", DC)
        xhat2T = ln_T(lambda dc, tt: x2T[dc][:, tt * 512 : (tt + 1) * 512],
                      TQ, V_GFFN, V_BEFFN, xhat2_p, "xhat2")

        # ================= phase 8: MLP =================
        w1_cm, w1_p = _pool(tc, "w1s", 3)
        w2_cm, w2_p = _pool(tc, "w2s", 3)
        h1_cm, h1_p = _pool(tc, "h1", NHC)
        out_cm, out_p = _pool(tc, "ostage", 4)
        w1r = w1.rearrange("(dc p) m -> p dc m", p=P)
        for tt in range(TQ // 512):
            sl = slice(tt * 512, (tt + 1) * 512)
            h1 = []
            for hc in range(NHC):
                w1t = w1_p.tile([P, DC, P], BF16, tag="w1",
                                name=f"w1_{tt}_{hc}")
                nc.sync.dma_start(w1t[:], w1r[:, :, hc * P : (hc + 1) * P])
                ph = ps.tile([P, 512], F32, tag="u", bufs=2,
                             name=f"ph1_{tt}_{hc}")
                for dc in range(DC):
                    nc.tensor.matmul(
                        ph[:], w1t[:, dc, :],
                        xhat2T[dc][:, sl], start=(dc == 0), stop=(dc == DC - 1),
                    )
                ht = h1_p.tile([P, 512], BF16, tag="h1", name=f"h1_{tt}_{hc}")
                nc.scalar.activation(ht[:], ph[:], AF.Gelu,
                                     bias=b1t[:, hc : hc + 1])
                h1.append(ht)
            for grp in range(2):   # fc groups of 4 (PSUM budget)
                pms = [ps.tile([P, 512], F32, tag="mm", bufs=4,
                               name=f"pmh2_{tt}_{grp}_{i}") for i in range(4)]
                for hc in range(NHC):
                    w2t = w2_p.tile([P, D], BF16, tag="w2",
                                    name=f"w2_{tt}_{grp}_{hc}")
                    nc.sync.dma_start(w2t[:], w2[hc * P : (hc + 1) * P, :])
                    for i in range(4):
                        fc = grp * 4 + i
                        nc.tensor.matmul(
                            pms[i][:], w2t[:, fc * P : (fc + 1) * P],
                            h1[hc][:], start=(hc == 0), stop=(hc == NHC - 1),
                        )
                for i in range(4):
                    fc = grp * 4 + i
                    ot = out_p.tile([P, 512], F32, tag="ostage",
                                    name=f"ot_{tt}_{grp}_{i}")
                    nc.vector.scalar_tensor_tensor(
                        ot[:], pms[i][:], gcol(V_B2, fc), x2T[fc][:, sl],
                        ALU.add, ALU.add,
                    )
                    nc.sync.dma_start(outT[fc * P : (fc + 1) * P, sl], ot[:])

        out_cm.__exit__(None, None, None)
        h1_cm.__exit__(None, None, None)
        w2_cm.__exit__(None, None, None)
        w1_cm.__exit__(None, None, None)
        xhat2_cm.__exit__(None, None, None)
        x2_cm.__exit__(None, None, None)
        stat_cm.__exit__(None, None, None)
        work_cm.__exit__(None, None, None)
        ps_cm.__exit__(None, None, None)
        const_cm.__exit__(None, None, None)

    nc.compile()
    return nc


def _col8(v):
    return np.ascontiguousarray(v.reshape(DC, P).T.astype(np.float32))


def make_in_maps(inputs):
    x = np.asarray(inputs["x"], np.float32)
    context = np.asarray(inputs["context"], np.float32)
    cos = np.asarray(inputs["rope_cos"], np.float32).reshape(L, HD)
    sin = np.asarray(inputs["rope_sin"], np.float32).reshape(L, HD)

    bf = lambda a: np.ascontiguousarray(np.asarray(a, np.float32)).astype(
        ml_dtypes.bfloat16)
    Wkv = np.asarray(inputs["Wkv"], np.float32)
    wq_b = bf(inputs["Wq"])
    wk_b = bf(Wkv[:, :D])
    wv_b = bf(Wkv[:, D:])
    wo_b = bf(inputs["Wo"])
    w1_b = bf(inputs["W1"])
    w2_b = bf(inputs["W2"])

    def _rot_col(v):
        # swap 32-halves within each 64-feature head block, per 128-chunk
        vv = np.asarray(v, np.float32).reshape(DC, 2, 2, 32)
        return np.ascontiguousarray(
            vv[:, :, ::-1, :].reshape(DC, P).T.astype(np.float32))

    bq_v = np.asarray(inputs["bq"], np.float32)
    bk_v = np.asarray(inputs["bkv"], np.float32)[:D]
    vecs = np.stack(
        [_col8(bq_v), _col8(bk_v)] +
        [_col8(np.asarray(inputs[k], np.float32)) for k in
         ["bo", "b2", "g_q", "be_q", "g_kv", "be_kv", "g_ffn", "be_ffn"]] +
        [_rot_col(bq_v), _rot_col(bk_v)],
        axis=1,
    )  # [128, 12, 8]
    vecs = np.ascontiguousarray(vecs)
    b1t = np.ascontiguousarray(
        np.asarray(inputs["b1"], np.float32).reshape(NHC, P).T)
    bvrow = np.ascontiguousarray(
        np.asarray(inputs["bkv"], np.float32)[D:].reshape(1, D))

    cosT = cos.T  # [64, 2048]
    sinT_rot = sin.T.copy()
    sinT_rot[0:32, :] = -sinT_rot[0:32, :]
    cosk_full = np.ascontiguousarray(np.concatenate([cosT, cosT], 0))
    sink_full = np.ascontiguousarray(np.concatenate([sinT_rot, sinT_rot], 0))

    in_maps = []
    for c in range(NCORES):
        b, hf = c // 2, c % 2
        tsl = slice(hf * TQ, (hf + 1) * TQ)
        in_maps.append({
            "xT": np.ascontiguousarray(x[b, tsl, :].T),
            "ctxT": np.ascontiguousarray(context[b].T),
            "cosq": np.ascontiguousarray(cosk_full[:, tsl]),
            "sinq": np.ascontiguousarray(sink_full[:, tsl]),
            "cosk": cosk_full,
            "sink": sink_full,
            "wq": wq_b, "wk": wk_b, "wv": wv_b, "wo": wo_b,
            "w1": w1_b, "w2": w2_b,
            "vecs": vecs, "b1t": b1t, "bvrow": bvrow,
        })
    return in_maps


def kernel(**inputs) -> np.ndarray:
    global _CACHED_NC
    if _CACHED_NC is None:
        _CACHED_NC = build_nc()
    nc = _CACHED_NC
    in_maps = make_in_maps(inputs)
    res = run_bass_kernel_spmd(nc, in_maps, core_ids=list(range(NCORES)))
    out = np.empty((B, L, D), np.float32)
    for c in range(NCORES):
        b, hf = c // 2, c % 2
        out[b, hf * TQ : (hf + 1) * TQ, :] = res.results[c]["outT"].T
    return out

